# revision 7
# baseline (speedup 1.0000x reference)
"""CrossAttention Trainium2 Bass kernel — 8 cores, batch-per-core sharding.

Per core b: all H=8 heads of batch b.
  q = (q_data @ Wq + bq) * c^-0.5        -> computed transposed qT [hc, S]
  k = m_data @ Wk                        -> kT [hc, K]
  v = m_data @ Wv                        -> natural layout [K, h*v] (+ ones col per head)
  sT[k,q] = k @ qT  (per head, contraction c=32, PE row-strip packed)
  p = exp(sT) * exp(bias).T              (bias folded via host-precomputed exp(bias))
  waT'[v+1, q] = sum_k v'[k, v+1] p[k, q]   (ones col -> softmax denominator in row 32)
  out[q, h, v] = waT[v, q].T * recip(den) * sigmoid(q_data @ Wg)

Key trick: exp(s+b) = exp(s)*exp(b); exp(b) precomputed on host (fp16),
so no on-chip bias add pass and the softmax denominator comes free from
the matmul ones-column.
"""
import numpy as np
from contextlib import ExitStack

import concourse.bass as bass
import concourse.tile as tile
from concourse import mybir
from concourse.bass_utils import run_bass_kernel_spmd
from concourse.masks import make_identity

F32 = mybir.dt.float32
F32R = mybir.dt.float32r
F16 = mybir.dt.float16

B, S, K, H, C, V, A = 8, 1024, 1024, 8, 32, 32, 256
HV = H * V            # 256
KEY_SCALE = C ** -0.5
N_CORES = 8
QT = S // 128         # 8 q tiles
KT = K // 128         # 8 k tiles


def _split_multi_waits(nc, max_waits=1):
    """walrus in this container allows only one semaphore wait per
    instruction; hoist extras onto same-engine nops inserted just before."""
    ctr = 0
    for fn in nc.m.functions:
        for blk in fn.blocks:
            insts = list(blk.instructions)
            out = []
            changed = False
            for inst in insts:
                si = inst.sync_info
                waits = list(si.on_wait) if (si is not None and si.on_wait) else []
                if len(waits) > max_waits:
                    changed = True
                    extra, keep = waits[:-max_waits], waits[-max_waits:]
                    for w in extra:
                        ctr += 1
                        nop = mybir.InstNoOp(
                            name=f"waitsplit_{ctr}",
                            engine=inst.engine,
                            ins=[],
                            outs=[],
                            sync_info=mybir.SyncInfo(on_wait=[w], on_update=[]),
                            bass_nofuse=True,
                        )
                        out.append(nop)
                    si.on_wait = keep
                out.append(inst)
            if changed:
                blk.set_instructions(out) if hasattr(blk, "set_instructions") else None
                if not hasattr(blk, "set_instructions"):
                    blk.instructions = out
    return ctr


def build():
    nc = bass.Bass()
    qT_d = nc.declare_dram_parameter("qT", [A, S], F16, isOutput=False)
    mT_d = nc.declare_dram_parameter("mT", [A, K], F16, isOutput=False)
    expb_d = nc.declare_dram_parameter("expb", [H, K, S], F16, isOutput=False)
    wq_d = nc.declare_dram_parameter("wq", [A, HV], F16, isOutput=False)
    wk_d = nc.declare_dram_parameter("wk", [A, HV], F16, isOutput=False)
    wv_d = nc.declare_dram_parameter("wv", [A, HV], F16, isOutput=False)
    wg_d = nc.declare_dram_parameter("wg", [A, HV], F16, isOutput=False)
    bq_d = nc.declare_dram_parameter("bq", [HV], F32, isOutput=False)
    out_d = nc.declare_dram_parameter("out", [S, HV], F32, isOutput=True)

    with tile.TileContext(nc) as tc, ExitStack() as ctx:
        singles = ctx.enter_context(tc.tile_pool(name="singles", bufs=1))
        es_pool = ctx.enter_context(tc.tile_pool(name="es", bufs=3))
        p_pool = ctx.enter_context(tc.tile_pool(name="pp", bufs=3))
        eb_pool = ctx.enter_context(tc.tile_pool(name="eb", bufs=4))
        wgs_pool = ctx.enter_context(tc.tile_pool(name="wgs", bufs=1))
        fin_pool = ctx.enter_context(tc.tile_pool(name="fin", bufs=4))
        ps_big = ctx.enter_context(tc.tile_pool(name="ps_big", bufs=2, space="PSUM"))
        ps_wa = ctx.enter_context(tc.tile_pool(name="ps_wa", bufs=1, space="PSUM"))
        ps_sm = ctx.enter_context(tc.tile_pool(name="ps_sm", bufs=2, space="PSUM"))

        # ---------- phase 0: load everything ----------
        qraw = singles.tile([128, 2, S], F16)       # [a-chunk part, chunk, q]
        mraw = singles.tile([128, 2, K], F16)
        for ac in range(2):
            nc.sync.dma_start(out=qraw[:, ac, :], in_=qT_d[ac * 128:(ac + 1) * 128, :])
            nc.sync.dma_start(out=mraw[:, ac, :], in_=mT_d[ac * 128:(ac + 1) * 128, :])
        wq_sb = singles.tile([128, 2, HV], F16)
        wk_sb = singles.tile([128, 2, HV], F16)
        wv_sb = singles.tile([128, 2, HV], F16)
        wg_sb = singles.tile([128, 2, HV], F16)
        for w_sb, w_d in ((wq_sb, wq_d), (wk_sb, wk_d), (wv_sb, wv_d), (wg_sb, wg_d)):
            for ac in range(2):
                nc.sync.dma_start(out=w_sb[:, ac, :], in_=w_d[ac * 128:(ac + 1) * 128, :])
        bq_sb = singles.tile([128, 2], F32)
        nc.sync.dma_start(out=bq_sb, in_=bq_d.rearrange("(h p) -> p h", p=128))
        ident = singles.tile([128, 128], F32)
        make_identity(nc, ident)

        # ---------- phase 1: projections ----------
        # gate[q, h*v] = sigmoid(q_data @ Wg), per q-tile (all heads packed)
        gate_sb = singles.tile([128, QT, HV], F32)
        for qt in range(QT):
            ps_g = ps_sm.tile([128, HV], F32, tag="ps_small")
            for ac in range(2):
                nc.tensor.matmul(ps_g, lhsT=qraw[:, ac, qt * 128:(qt + 1) * 128],
                                 rhs=wg_sb[:, ac, :], start=(ac == 0), stop=(ac == 1))
            nc.scalar.activation(gate_sb[:, qt, :], ps_g,
                                 mybir.ActivationFunctionType.Sigmoid)

        # qT_all / kT_all: [hc(4 heads), S] per half, scaled+biased q
        qT_sb = singles.tile([128, 2, S], F16)
        kT_sb = singles.tile([128, 2, K], F16)
        for half in range(2):
            for qh in range(2):
                ps_q = ps_big.tile([128, 512], F32, tag="ps_big")
                for ac in range(2):
                    nc.tensor.matmul(ps_q,
                                     lhsT=wq_sb[:, ac, half * 128:(half + 1) * 128],
                                     rhs=qraw[:, ac, qh * 512:(qh + 1) * 512],
                                     start=(ac == 0), stop=(ac == 1))
                nc.vector.tensor_scalar(
                    qT_sb[:, half, qh * 512:(qh + 1) * 512], ps_q,
                    KEY_SCALE, bq_sb[:, half:half + 1],
                    mybir.AluOpType.mult, mybir.AluOpType.add)
                ps_k = ps_big.tile([128, 512], F32, tag="ps_big")
                for ac in range(2):
                    nc.tensor.matmul(ps_k,
                                     lhsT=wk_sb[:, ac, half * 128:(half + 1) * 128],
                                     rhs=mraw[:, ac, qh * 512:(qh + 1) * 512],
                                     start=(ac == 0), stop=(ac == 1))
                nc.vector.tensor_copy(out=kT_sb[:, half, qh * 512:(qh + 1) * 512],
                                      in_=ps_k)

        # v natural layout + ones column: [k-tile part, h, v+1] fp16
        v_sb = singles.tile([128, KT, H, V + 1], F16)
        nc.gpsimd.memset(v_sb, 1.0)
        for kt in range(KT):
            ps_v = ps_sm.tile([128, HV], F32, tag="ps_small")
            for ac in range(2):
                nc.tensor.matmul(ps_v, lhsT=mraw[:, ac, kt * 128:(kt + 1) * 128],
                                 rhs=wv_sb[:, ac, :], start=(ac == 0), stop=(ac == 1))
            nc.vector.tensor_copy(
                out=v_sb[:, kt, :, 0:V],
                in_=ps_v.rearrange("p (h c) -> p h c", c=V))

        # ---------- phase 2: per-head attention + fused finalize ----------
        out_sb = singles.tile([128, QT, HV], F32)
        for h in range(H):
            half, strip = h // 4, (h % 4) * 32
            ps_wa_t = ps_wa.tile([33, S], F32, tag="ps_wa")
            for kt in range(KT):
                ps_s = ps_big.tile([128, S], F32, tag="ps_big")
                for qh in range(2):
                    nc.tensor.matmul(
                        ps_s[:, qh * 512:(qh + 1) * 512],
                        lhsT=kT_sb[strip:strip + 32, half, kt * 128:(kt + 1) * 128],
                        rhs=qT_sb[strip:strip + 32, half, qh * 512:(qh + 1) * 512],
                        start=True, stop=True,
                        tile_position=(strip, 0))
                es = es_pool.tile([128, S], F16, tag="es")
                nc.scalar.activation(es, ps_s, mybir.ActivationFunctionType.Exp)
                eb = eb_pool.tile([128, S], F16, tag="eb")
                nc.sync.dma_start(out=eb, in_=expb_d[h, kt * 128:(kt + 1) * 128, :])
                p = p_pool.tile([128, S], F16, tag="p")
                nc.vector.tensor_mul(out=p, in0=es, in1=eb)
                for qh in range(2):
                    nc.tensor.matmul(
                        ps_wa_t[:, qh * 512:(qh + 1) * 512],
                        lhsT=v_sb[:, kt, h, :],
                        rhs=p[:, qh * 512:(qh + 1) * 512],
                        start=(kt == 0), stop=(kt == KT - 1))
            # finalize this head while the next head computes
            wgt = wgs_pool.tile([33, S], F32, tag="wgt", bufs=2)
            nc.vector.tensor_copy(out=wgt, in_=ps_wa_t)
            ps_t = ps_sm.tile([128, QT, V + 1], F32, tag="ps_small")
            for qt in range(QT):
                nc.tensor.transpose(ps_t[:, qt, :],
                                    wgt[:, qt * 128:(qt + 1) * 128],
                                    ident[0:33, 0:33])
            d_sb = fin_pool.tile([128, QT], F32, tag="d")
            nc.vector.tensor_copy(out=d_sb, in_=ps_t[:, :, V])
            r_sb = fin_pool.tile([128, QT], F32, tag="r")
            nc.vector.reciprocal(out=r_sb, in_=d_sb)
            rg_sb = fin_pool.tile([128, QT, V], F32, tag="rg")
            for qt in range(QT):
                nc.vector.tensor_scalar_mul(
                    rg_sb[:, qt, :],
                    gate_sb[:, qt, h * V:(h + 1) * V],
                    r_sb[:, qt:qt + 1])
            nc.vector.tensor_mul(
                out=out_sb.rearrange("p q (h c) -> p q h c", c=V)[:, :, h, :],
                in0=ps_t[:, :, 0:V],
                in1=rg_sb)

        # ---------- phase 3: store ----------
        for qt in range(QT):
            nc.sync.dma_start(out=out_d[qt * 128:(qt + 1) * 128, :],
                              in_=out_sb[:, qt, :])

    n = _split_multi_waits(nc)
    return nc


_NC = None


def _get_nc():
    global _NC
    if _NC is None:
        _NC = build()
    return _NC


def _make_in_maps(q_data, m_data, batched_bias, query_w, query_b, key_w,
                  value_w, gating_w):
    q_data = np.asarray(q_data, dtype=np.float32)
    m_data = np.asarray(m_data, dtype=np.float32)
    batched_bias = np.asarray(batched_bias, dtype=np.float32)
    wq = np.ascontiguousarray(np.asarray(query_w, np.float32).reshape(A, HV)).astype(np.float16)
    wk = np.ascontiguousarray(np.asarray(key_w, np.float32).reshape(A, HV)).astype(np.float16)
    wv = np.ascontiguousarray(np.asarray(value_w, np.float32).reshape(A, HV)).astype(np.float16)
    wg = np.ascontiguousarray(np.asarray(gating_w, np.float32).reshape(A, HV)).astype(np.float16)
    bq = np.ascontiguousarray(
        (np.asarray(query_b, np.float32) * KEY_SCALE).reshape(HV))
    in_maps = []
    for b in range(N_CORES):
        expb = np.exp(batched_bias[b].transpose(0, 2, 1)).astype(np.float16)
        in_maps.append({
            "qT": np.ascontiguousarray(q_data[b].T).astype(np.float16),
            "mT": np.ascontiguousarray(m_data[b].T).astype(np.float16),
            "expb": np.ascontiguousarray(expb),
            "wq": wq, "wk": wk, "wv": wv, "wg": wg, "bq": bq,
        })
    return in_maps


def run_spmd(in_maps, **kw):
    nc = _get_nc()
    return run_bass_kernel_spmd(nc, in_maps, list(range(N_CORES)), **kw)


def kernel(q_data, m_data, batched_bias, query_w, query_b, key_w, value_w,
           gating_w):
    in_maps = _make_in_maps(q_data, m_data, batched_bias, query_w, query_b,
                            key_w, value_w, gating_w)
    res = run_spmd(in_maps)
    out = np.stack([res.results[b]["out"] for b in range(N_CORES)])
    return out.reshape(B, S, H, V).astype(np.float32)
